# revision 8
# baseline (speedup 1.0000x reference)
"""GQA attention (B=2, S=2048, dm=1024, 16 Q heads / 4 KV heads, dh=64, RoPE,
causal) on 8 trn2 NeuronCores.

Sharding: core c = (b, g) with b = c // 4 (batch), g = c % 4 (KV group).
Each core computes its 4 Q heads + 1 KV head end-to-end (flash-style) plus its
partial Wo projection; host sums the 4 partials per batch element.

v3: bf16 matmul operands throughout (PE streams 1 col/cycle vs <1/2 for
fp32r), transposed layouts:
  x^T [dm,S] -> Q^T stored [64, 4head, S], K^T [64, S], V tiles [k,128]
  (cols 0:64 = V, 64:128 = 1.0 so the ctx matmul's rows 64:128 deliver the
  softmax denominator pre-broadcast across partitions). P^T = exp(S^T/8)
  with causal column restriction; only the [128,128] diagonal triangle is
  masked (gpsimd). 1/denom via DVE reciprocal_approx_fast (SBUF-staged).
The attention inner loop is scalar-exp bound, so projection/Wo matmuls are
interleaved as "fillers" between k-tile steps to keep the PE busy and the
HAM clock gate warm.
"""

import os
import sys
from collections import deque

import numpy as np
import ml_dtypes

try:
    from concourse import bass_utils
except ImportError:
    for _p in ("/opt/trn_rl_repo", "/root/.axon_site/_ro/trn_rl_repo"):
        if os.path.isdir(_p) and _p not in sys.path:
            sys.path.insert(0, _p)
    from concourse import bass_utils

import concourse.bass as bass
import concourse.mybir as mybir
import concourse.tile as tile
from concourse import bacc

F32 = mybir.dt.float32
BF = mybir.dt.bfloat16
EXP = mybir.ActivationFunctionType.Exp
COPY = mybir.ActivationFunctionType.Copy
MULT = mybir.AluOpType.mult
ADD = mybir.AluOpType.add

B, S, DM = 2, 2048, 1024
H, KV, DH = 16, 4, 64
HPG = H // KV          # 4 q-heads per kv group (per core)
DG = HPG * DH          # 256 local q dims
NCHIPS = 8
QB = 512               # q block width
KT = 128               # k tile width
NKT = S // KT          # 16 k tiles
NCH = S // QB          # 4 chunks == 4 q blocks

BF_NP = ml_dtypes.bfloat16


def build_bass():
    nc = bacc.Bacc()
    xT = nc.declare_dram_parameter("xT", [DM, S], BF, isOutput=False)
    wq = nc.declare_dram_parameter("wq", [DM, DG], BF, isOutput=False)
    wk = nc.declare_dram_parameter("wk", [DM, DH], BF, isOutput=False)
    wv = nc.declare_dram_parameter("wv", [DM, DH], BF, isOutput=False)
    wo = nc.declare_dram_parameter("wo", [DG, DM], BF, isOutput=False)
    cos2 = nc.declare_dram_parameter("cos2", [128, S], F32, isOutput=False)
    sin2 = nc.declare_dram_parameter("sin2", [128, S], F32, isOutput=False)
    ident = nc.declare_dram_parameter("ident", [128, 128], BF, isOutput=False)
    mask2 = nc.declare_dram_parameter("mask2", [128, 2, 128], BF, isOutput=False)
    out = nc.declare_dram_parameter("out", [S, DM], F32, isOutput=True)

    from contextlib import ExitStack
    with tile.TileContext(nc) as tc, ExitStack() as es:
        cst = es.enter_context(tc.tile_pool(name="cst", bufs=1))
        sbQ = es.enter_context(tc.tile_pool(name="sbQ", bufs=1))
        sbX = es.enter_context(tc.tile_pool(name="sbX", bufs=2))
        sbR = es.enter_context(tc.tile_pool(name="sbR", bufs=2))
        sbP = es.enter_context(tc.tile_pool(name="sbP", bufs=4))
        sbN = es.enter_context(tc.tile_pool(name="sbN", bufs=2))
        sbO = es.enter_context(tc.tile_pool(name="sbO", bufs=4))
        ps = es.enter_context(tc.tile_pool(name="ps", bufs=2, space="PSUM"))

        # ---- weights first, then chunk 0 of x, then the rest -------------
        wq_sb = cst.tile([128, 8, DG], BF)
        nc.sync.dma_start(wq_sb[:], wq.rearrange("(a p) n -> p a n", p=128))
        wk_sb = cst.tile([128, 8, DH], BF)
        nc.sync.dma_start(wk_sb[:], wk.rearrange("(a p) n -> p a n", p=128))
        wv_sb = cst.tile([128, 8, DH], BF)
        nc.sync.dma_start(wv_sb[:], wv.rearrange("(a p) n -> p a n", p=128))

        xTr = xT.rearrange("(a p) s -> p a s", p=128)
        xcs = [sbX.tile([128, 8, QB], BF, tag="xc", name=f"xc{i}")
               for i in range(NCH)]
        nc.sync.dma_start(xcs[0][:], xTr[:, :, 0:QB])

        cos_sb = cst.tile([128, S], F32)
        sin_sb = cst.tile([128, S], F32)
        nc.sync.dma_start(cos_sb[:, 0:QB], cos2[:, 0:QB])
        nc.sync.dma_start(sin_sb[:, 0:QB], sin2[:, 0:QB])
        id_sb = cst.tile([128, 128], BF)
        nc.sync.dma_start(id_sb[:], ident[:])
        mask_sb = cst.tile([128, 2, 128], BF)
        nc.sync.dma_start(mask_sb[:], mask2[:])
        nc.sync.dma_start(xcs[1][:], xTr[:, :, QB:2 * QB])
        wo_sb = cst.tile([128, 2, DM], BF)
        nc.sync.dma_start(wo_sb[:], wo.rearrange("(a p) n -> p a n", p=128))
        for c in range(1, NCH):
            nc.sync.dma_start(cos_sb[:, c * QB:(c + 1) * QB],
                              cos2[:, c * QB:(c + 1) * QB])
            nc.sync.dma_start(sin_sb[:, c * QB:(c + 1) * QB],
                              sin2[:, c * QB:(c + 1) * QB])

        # persistent activations
        qT4 = sbQ.tile([DH, HPG, S], BF)             # Q^T, head-major cols
        kT = sbQ.tile([DH, S], BF)                   # K^T
        vt = [sbQ.tile([128, 128], BF, tag=f"v{i}", name=f"v{i}")
              for i in range(NKT)]
        for i in range(NKT):
            nc.gpsimd.memset(vt[i][:, DH:128], 1.0)  # denominator columns
        packed = sbQ.tile([128, 2, S], BF)           # normalized ctx^T

        def rope(dst, qp, rows, c0):
            """dst[...] (bf16) = qp[0:rows]*cos + rotate_half(qp[0:rows])*sin.

            qp is fp32 PSUM; rows is 64 or 128 (1 or 2 stacked heads);
            dst is a list of per-64-row destination APs ([64, QB] each).
            """
            rot = sbR.tile([128, QB], F32, tag="rot")
            for h0 in range(0, rows, DH):
                nc.scalar.activation(rot[h0:h0 + 32, :], qp[h0 + 32:h0 + 64, :],
                                     COPY, scale=-1.0)
                nc.scalar.activation(rot[h0 + 32:h0 + 64, :], qp[h0:h0 + 32, :],
                                     COPY, scale=1.0)
            t1 = sbR.tile([128, QB], BF, tag="t1")
            nc.vector.tensor_tensor(t1[0:rows, :], qp[0:rows, :],
                                    cos_sb[0:rows, c0:c0 + QB], MULT)
            rs = sbR.tile([128, QB], BF, tag="rs")
            nc.vector.tensor_tensor(rs[0:rows, :], rot[0:rows, :],
                                    sin_sb[0:rows, c0:c0 + QB], MULT)
            for i, d in enumerate(dst):
                nc.vector.tensor_tensor(
                    d, t1[i * DH:(i + 1) * DH, :], rs[i * DH:(i + 1) * DH, :],
                    ADD)

        def proj_fillers(ch):
            """Closures emitting the projections+rope for chunk ch."""
            c0 = ch * QB
            xc = xcs[ch]
            fl = []
            state = {}

            def qchain(mt, lo, hi):
                def go():
                    if lo == 0:
                        state[mt] = ps.tile([128, QB], F32, tag="pa",
                                            name=f"qp{ch}{mt}", uniquify=True)
                    qp = state[mt]
                    for ki in range(lo, hi):
                        nc.tensor.matmul(
                            qp[:], wq_sb[:, ki, mt * 128:mt * 128 + 128],
                            xc[:, ki, :], start=(ki == 0), stop=(ki == 7))
                return go

            def qrope(mt):
                def go():
                    rope([qT4[:, 2 * mt, c0:c0 + QB],
                          qT4[:, 2 * mt + 1, c0:c0 + QB]], state[mt], 128, c0)
                return go

            for mt in range(2):
                for lo in range(0, 8, 3):
                    fl.append(qchain(mt, lo, min(lo + 3, 8)))
                fl.append(qrope(mt))

            def kchain(lo, hi):
                def go():
                    if lo == 0:
                        state['k'] = ps.tile([128, QB], F32, tag="pa",
                                             name=f"kp{ch}", uniquify=True)
                    kp = state['k']
                    for ki in range(lo, hi):
                        nc.tensor.matmul(kp[0:DH, :], wk_sb[:, ki, :],
                                         xc[:, ki, :],
                                         start=(ki == 0), stop=(ki == 7))
                return go

            for lo in range(0, 8, 3):
                fl.append(kchain(lo, min(lo + 3, 8)))
            fl.append(lambda: rope([kT[:, c0:c0 + QB]], state['k'], DH, c0))

            def vchain(lo, hi):
                def go():
                    if lo == 0:
                        state['v'] = ps.tile([128, QB], F32, tag="pa",
                                             name=f"vp{ch}", uniquify=True)
                    vp = state['v']
                    for ki in range(lo, hi):
                        nc.tensor.matmul(vp[0:DH, :], wv_sb[:, ki, :],
                                         xc[:, ki, :],
                                         start=(ki == 0), stop=(ki == 7))
                return go

            for lo in range(0, 8, 3):
                fl.append(vchain(lo, min(lo + 3, 8)))

            def vmove():
                vTs = sbR.tile([DH, QB], BF, tag="vT")
                nc.scalar.activation(vTs[:], state['v'][0:DH, :], COPY)
                state['vT'] = vTs
            fl.append(vmove)

            def vtrans(sub):
                def go():
                    tp = ps.tile([128, DH], BF, tag="pa", name=f"tp{ch}{sub}",
                                 uniquify=True)
                    nc.tensor.transpose(
                        tp[:], state['vT'][:, sub * 128:sub * 128 + 128],
                        id_sb[0:DH, 0:DH])
                    nc.vector.tensor_copy(vt[ch * 4 + sub][:, 0:DH], tp[:])
                return go

            for sub in range(QB // 128):
                fl.append(vtrans(sub))
            return fl

        def wo_fillers(jb):
            """Closures emitting the Wo projection for q block jb."""
            q0 = jb * QB
            fl = []

            def wostep(st, nb):
                def go():
                    s0 = q0 + st * 128
                    op = ps.tile([128, QB], F32, tag="pa",
                                 name=f"op{jb}{st}{nb}", uniquify=True)
                    for kt2 in range(2):
                        nc.tensor.matmul(
                            op[:], packed[:, kt2, s0:s0 + 128],
                            wo_sb[:, kt2, nb * QB:nb * QB + QB],
                            start=(kt2 == 0), stop=(kt2 == 1))
                    ot = sbO.tile([128, QB], F32, tag="ot")
                    nc.vector.tensor_copy(ot[:], op[:])
                    nc.sync.dma_start(
                        out[s0:s0 + 128, nb * QB:nb * QB + QB], ot[:])
                return go

            for st in range(QB // 128):
                for nb in range(2):
                    fl.append(wostep(st, nb))
            return fl

        fillers = deque()

        def drain(n):
            for _ in range(n):
                if not fillers:
                    return
                fillers.popleft()()

        # chunk 0 projections run standalone (pipeline head)
        for f in proj_fillers(0):
            f()

        for ch in range(NCH):
            jb = ch
            q0 = jb * QB
            nkt = 4 * (jb + 1)
            # prefetch x chunks, queue next projections + previous Wo
            if ch + 2 < NCH:
                nc.sync.dma_start(xcs[ch + 2][:],
                                  xTr[:, :, (ch + 2) * QB:(ch + 3) * QB])
            if ch + 1 < NCH:
                fillers.extend(proj_fillers(ch + 1))
            if ch >= 1:
                fillers.extend(wo_fillers(ch - 1))
            steps_left = 2 * nkt
            for pair in range(2):
                cps = [ps.tile([128, QB], F32, tag="cp", name=f"cp{jb}{pair}{hh}")
                       for hh in range(2)]

                def score_step(kt_i):
                    """scores + exp + mask for one k tile (one step ahead of
                    the ctx matmuls so the PE never sits behind ScalarE)."""
                    cc = max(0, (kt_i - 4 * jb) * KT)
                    sp = ps.tile([128, 2, QB], F32, tag="sp", name="sp",
                                 uniquify=True)
                    for hh in range(2):
                        h = 2 * pair + hh
                        nc.tensor.matmul(
                            sp[:, hh, cc:], kT[:, kt_i * KT:kt_i * KT + KT],
                            qT4[:, h, q0 + cc:q0 + QB], start=True, stop=True)
                    pt = sbP.tile([128, 2, QB], BF, tag="pt", name="pt",
                                  uniquify=True)
                    nc.scalar.activation(pt[:, :, cc:], sp[:, :, cc:], EXP,
                                         scale=0.125)
                    if kt_i >= 4 * jb:
                        nc.gpsimd.tensor_tensor(
                            pt[:, :, cc:cc + KT], pt[:, :, cc:cc + KT],
                            mask_sb[:], MULT)
                    return pt

                pts = {0: score_step(0)}
                for kt_i in range(nkt):
                    if kt_i + 1 < nkt:
                        pts[kt_i + 1] = score_step(kt_i + 1)
                    cc = max(0, (kt_i - 4 * jb) * KT)
                    pt = pts.pop(kt_i)
                    for hh in range(2):
                        nc.tensor.matmul(
                            cps[hh][:, cc:], vt[kt_i][:], pt[:, hh, cc:],
                            start=(kt_i == 0), stop=(kt_i == nkt - 1))
                    # keep the PE busy while ScalarE runs exp
                    drain(-(-len(fillers) // steps_left))
                    steps_left -= 1
                # normalize: 1/denom (rows 64:128) * ctx (rows 0:64)
                for hh in range(2):
                    h = 2 * pair + hh
                    dns = sbN.tile([DH, QB], F32, tag="dns")
                    nc.vector.tensor_copy(dns[:], cps[hh][DH:128, :])
                    rb = sbN.tile([DH, QB], F32, tag="rb")
                    nc.vector.reciprocal_approx_fast(rb[:], dns[:])
                    p0 = (h % 2) * DH
                    nc.vector.tensor_tensor(
                        packed[p0:p0 + DH, h // 2, q0:q0 + QB],
                        cps[hh][0:DH, :], rb[:], MULT)
            drain(len(fillers))
        for f in wo_fillers(NCH - 1):
            f()
    nc.compile()
    return nc


def _rope_tables():
    inv = 1.0 / (10000.0 ** (np.arange(0, DH, 2, dtype=np.float32) / DH))
    t = np.arange(S, dtype=np.float32)
    freqs = np.outer(t, inv).astype(np.float32)          # [S, 32]
    emb = np.concatenate([freqs, freqs], axis=1)         # [S, 64]
    cosT = np.cos(emb).T.astype(np.float32)              # [64, S]
    sinT = np.sin(emb).T.astype(np.float32)
    return (np.concatenate([cosT, cosT], 0).copy(),
            np.concatenate([sinT, sinT], 0).copy())


def _mask2():
    r = np.arange(128)[:, None]
    c = np.arange(128)[None, :]
    m = (r <= c).astype(np.float32)
    return np.broadcast_to(m[:, None, :], (128, 2, 128)).astype(BF_NP).copy()


_NC_CACHE = {}


def _get_nc():
    if "nc" not in _NC_CACHE:
        _NC_CACHE["nc"] = build_bass()
    return _NC_CACHE["nc"]


def run(x, Wq, Wk, Wv, Wo, trace=False):
    nc = _get_nc()
    cos2, sin2 = _rope_tables()
    ident = np.eye(128, dtype=np.float32).astype(BF_NP)
    mask = _mask2()
    in_maps = []
    for c in range(NCHIPS):
        b, g = c // KV, c % KV
        in_maps.append({
            "xT": np.ascontiguousarray(x[b].T).astype(BF_NP),
            "wq": np.ascontiguousarray(Wq[:, g * DG:(g + 1) * DG]).astype(BF_NP),
            "wk": np.ascontiguousarray(Wk[:, g * DH:(g + 1) * DH]).astype(BF_NP),
            "wv": np.ascontiguousarray(Wv[:, g * DH:(g + 1) * DH]).astype(BF_NP),
            "wo": np.ascontiguousarray(Wo[g * DG:(g + 1) * DG, :]).astype(BF_NP),
            "cos2": cos2, "sin2": sin2, "ident": ident, "mask2": mask,
        })
    res = bass_utils.run_bass_kernel_spmd(
        nc, in_maps, core_ids=list(range(NCHIPS)), trace=trace)
    outs = [np.asarray(r["out"], dtype=np.float32) for r in res.results]
    full = np.zeros((B, S, DM), dtype=np.float32)
    for c in range(NCHIPS):
        full[c // KV] += outs[c]
    return full, res


def kernel(x, Wq, Wk, Wv, Wo):
    full, _ = run(np.asarray(x, dtype=np.float32), np.asarray(Wq),
                  np.asarray(Wk), np.asarray(Wv), np.asarray(Wo))
    return full


# revision 12
# speedup vs baseline: 1.0279x; 1.0279x over previous
"""GQA attention (B=2, S=2048, dm=1024, 16 Q heads / 4 KV heads, dh=64, RoPE,
causal) on 8 trn2 NeuronCores.

Sharding: core c = (b, g) with b = c // 4 (batch), g = c % 4 (KV group).
Each core computes its 4 Q heads + 1 KV head end-to-end (flash-style) plus its
partial Wo projection; host sums the 4 partials per batch element.

v3: bf16 matmul operands throughout (PE streams 1 col/cycle vs <1/2 for
fp32r), transposed layouts:
  x^T [dm,S] -> Q^T stored [64, 4head, S], K^T [64, S], V tiles [k,128]
  (cols 0:64 = V, 64:128 = 1.0 so the ctx matmul's rows 64:128 deliver the
  softmax denominator pre-broadcast across partitions). P^T = exp(S^T/8)
  with causal column restriction; only the [128,128] diagonal triangle is
  masked (gpsimd). 1/denom via DVE reciprocal_approx_fast (SBUF-staged).
The attention inner loop is scalar-exp bound, so projection/Wo matmuls are
interleaved as "fillers" between k-tile steps to keep the PE busy and the
HAM clock gate warm.
"""

import os
import sys
from collections import deque

import numpy as np
import ml_dtypes

try:
    from concourse import bass_utils
except ImportError:
    for _p in ("/opt/trn_rl_repo", "/root/.axon_site/_ro/trn_rl_repo"):
        if os.path.isdir(_p) and _p not in sys.path:
            sys.path.insert(0, _p)
    from concourse import bass_utils

import concourse.bass as bass
import concourse.mybir as mybir
import concourse.tile as tile
from concourse import bacc

F32 = mybir.dt.float32
BF = mybir.dt.bfloat16
EXP = mybir.ActivationFunctionType.Exp
COPY = mybir.ActivationFunctionType.Copy
MULT = mybir.AluOpType.mult
ADD = mybir.AluOpType.add

B, S, DM = 2, 2048, 1024
H, KV, DH = 16, 4, 64
HPG = H // KV          # 4 q-heads per kv group (per core)
DG = HPG * DH          # 256 local q dims
NCHIPS = 8
QB = 512               # q block width
KT = 128               # k tile width
NKT = S // KT          # 16 k tiles
NCH = S // QB          # 4 chunks == 4 q blocks

BF_NP = ml_dtypes.bfloat16


def build_bass():
    nc = bacc.Bacc()
    xT = nc.declare_dram_parameter("xT", [DM, S], BF, isOutput=False)
    wq = nc.declare_dram_parameter("wq", [DM, DG], BF, isOutput=False)
    wk = nc.declare_dram_parameter("wk", [DM, DH], BF, isOutput=False)
    wv = nc.declare_dram_parameter("wv", [DM, DH], BF, isOutput=False)
    wo = nc.declare_dram_parameter("wo", [DG, DM], BF, isOutput=False)
    cos2 = nc.declare_dram_parameter("cos2", [128, S], F32, isOutput=False)
    sin2 = nc.declare_dram_parameter("sin2", [128, S], F32, isOutput=False)
    ident = nc.declare_dram_parameter("ident", [128, 128], BF, isOutput=False)
    mask2 = nc.declare_dram_parameter("mask2", [128, 2, 128], BF, isOutput=False)
    out = nc.declare_dram_parameter("out", [S, DM], F32, isOutput=True)

    from contextlib import ExitStack
    with tile.TileContext(nc) as tc, ExitStack() as es:
        cst = es.enter_context(tc.tile_pool(name="cst", bufs=1))
        sbQ = es.enter_context(tc.tile_pool(name="sbQ", bufs=1))
        sbX = es.enter_context(tc.tile_pool(name="sbX", bufs=2))
        sbR = es.enter_context(tc.tile_pool(name="sbR", bufs=2))
        sbP = es.enter_context(tc.tile_pool(name="sbP", bufs=4))
        sbN = es.enter_context(tc.tile_pool(name="sbN", bufs=2))
        sbO = es.enter_context(tc.tile_pool(name="sbO", bufs=4))
        ps = es.enter_context(tc.tile_pool(name="ps", bufs=2, space="PSUM"))

        # ---- weights first, then chunk 0 of x, then the rest -------------
        wq_sb = cst.tile([128, 8, DG], BF)
        nc.sync.dma_start(wq_sb[:], wq.rearrange("(a p) n -> p a n", p=128))
        wk_sb = cst.tile([128, 8, DH], BF)
        nc.sync.dma_start(wk_sb[:], wk.rearrange("(a p) n -> p a n", p=128))
        wv_sb = cst.tile([128, 8, DH], BF)
        nc.sync.dma_start(wv_sb[:], wv.rearrange("(a p) n -> p a n", p=128))

        xTr = xT.rearrange("(a p) s -> p a s", p=128)
        xcs = [sbX.tile([128, 8, QB], BF, tag="xc", name=f"xc{i}")
               for i in range(NCH)]
        nc.sync.dma_start(xcs[0][:], xTr[:, :, 0:QB])

        cos_sb = cst.tile([128, S], F32)
        sin_sb = cst.tile([128, S], F32)
        nc.sync.dma_start(cos_sb[:, 0:QB], cos2[:, 0:QB])
        nc.sync.dma_start(sin_sb[:, 0:QB], sin2[:, 0:QB])
        id_sb = cst.tile([128, 128], BF)
        nc.sync.dma_start(id_sb[:], ident[:])
        mask_sb = cst.tile([128, 2, 128], BF)
        nc.sync.dma_start(mask_sb[:], mask2[:])
        nc.sync.dma_start(xcs[1][:], xTr[:, :, QB:2 * QB])
        wo_sb = cst.tile([128, 2, DM], BF)
        nc.sync.dma_start(wo_sb[:], wo.rearrange("(a p) n -> p a n", p=128))
        for c in range(1, NCH):
            nc.sync.dma_start(cos_sb[:, c * QB:(c + 1) * QB],
                              cos2[:, c * QB:(c + 1) * QB])
            nc.sync.dma_start(sin_sb[:, c * QB:(c + 1) * QB],
                              sin2[:, c * QB:(c + 1) * QB])

        # persistent activations
        qT2 = sbQ.tile([128, 2, S], BF)              # Q^T, head pairs stacked
        kT = sbQ.tile([128, S], BF)                  # K^T duplicated rows
        vt = [sbQ.tile([128, 128], BF, tag=f"v{i}", name=f"v{i}")
              for i in range(NKT)]
        for i in range(NKT):
            nc.gpsimd.memset(vt[i][:, DH:128], 1.0)  # denominator columns
        packed = sbQ.tile([128, 2, S], BF)           # normalized ctx^T

        def rope(dsts, qp, rows, c0):
            """dst (bf16) = qp[0:rows]*cos + rotate_half(qp[0:rows])*sin.

            qp is fp32 PSUM; rows is 64 or 128 (1 or 2 stacked heads);
            dsts is a list of (ap, src_row_lo, n_rows) destinations.
            """
            rot = sbR.tile([128, QB], F32, tag="rot")
            for h0 in range(0, rows, DH):
                nc.scalar.activation(rot[h0:h0 + 32, :], qp[h0 + 32:h0 + 64, :],
                                     COPY, scale=-1.0)
                nc.scalar.activation(rot[h0 + 32:h0 + 64, :], qp[h0:h0 + 32, :],
                                     COPY, scale=1.0)
            t1 = sbR.tile([128, QB], BF, tag="t1")
            nc.vector.tensor_tensor(t1[0:rows, :], qp[0:rows, :],
                                    cos_sb[0:rows, c0:c0 + QB], MULT)
            rs = sbR.tile([128, QB], BF, tag="rs")
            nc.vector.tensor_tensor(rs[0:rows, :], rot[0:rows, :],
                                    sin_sb[0:rows, c0:c0 + QB], MULT)
            for d, lo, nr in dsts:
                nc.vector.tensor_tensor(
                    d, t1[lo:lo + nr, :], rs[lo:lo + nr, :], ADD)

        def proj_fillers(ch):
            """Closures emitting the projections+rope for chunk ch."""
            c0 = ch * QB
            xc = xcs[ch]
            fl = []
            state = {}

            def qchain(mt, lo, hi):
                def go():
                    if lo == 0:
                        state[mt] = ps.tile([128, QB], F32, tag="pa",
                                            name=f"qp{ch}{mt}", uniquify=True)
                    qp = state[mt]
                    for ki in range(lo, hi):
                        nc.tensor.matmul(
                            qp[:], wq_sb[:, ki, mt * 128:mt * 128 + 128],
                            xc[:, ki, :], start=(ki == 0), stop=(ki == 7))
                return go

            def qrope(mt):
                def go():
                    rope([(qT2[:, mt, c0:c0 + QB], 0, 128)], state[mt], 128, c0)
                return go

            for mt in range(2):
                for lo in range(0, 8, 3):
                    fl.append(qchain(mt, lo, min(lo + 3, 8)))
                fl.append(qrope(mt))

            def kchain(lo, hi):
                def go():
                    if lo == 0:
                        state['k'] = ps.tile([128, QB], F32, tag="pa",
                                             name=f"kp{ch}", uniquify=True)
                    kp = state['k']
                    for ki in range(lo, hi):
                        nc.tensor.matmul(kp[0:DH, :], wk_sb[:, ki, :],
                                         xc[:, ki, :],
                                         start=(ki == 0), stop=(ki == 7))
                return go

            for lo in range(0, 8, 3):
                fl.append(kchain(lo, min(lo + 3, 8)))
            fl.append(lambda: rope([(kT[0:DH, c0:c0 + QB], 0, DH),
                                    (kT[DH:128, c0:c0 + QB], 0, DH)],
                                   state['k'], DH, c0))

            def vchain(lo, hi):
                def go():
                    if lo == 0:
                        state['v'] = ps.tile([128, QB], F32, tag="pa",
                                             name=f"vp{ch}", uniquify=True)
                    vp = state['v']
                    for ki in range(lo, hi):
                        nc.tensor.matmul(vp[0:DH, :], wv_sb[:, ki, :],
                                         xc[:, ki, :],
                                         start=(ki == 0), stop=(ki == 7))
                return go

            for lo in range(0, 8, 3):
                fl.append(vchain(lo, min(lo + 3, 8)))

            def vmove():
                vTs = sbR.tile([DH, QB], BF, tag="vT")
                nc.scalar.activation(vTs[:], state['v'][0:DH, :], COPY)
                state['vT'] = vTs
            fl.append(vmove)

            def vtrans(sub):
                def go():
                    tp = ps.tile([128, DH], BF, tag="pa", name=f"tp{ch}{sub}",
                                 uniquify=True)
                    nc.tensor.transpose(
                        tp[:], state['vT'][:, sub * 128:sub * 128 + 128],
                        id_sb[0:DH, 0:DH])
                    nc.vector.tensor_copy(vt[ch * 4 + sub][:, 0:DH], tp[:])
                return go

            for sub in range(QB // 128):
                fl.append(vtrans(sub))
            return fl

        def wo_fillers(jb):
            """Closures emitting the Wo projection for q block jb."""
            q0 = jb * QB
            fl = []

            def wostep(st, nb):
                def go():
                    s0 = q0 + st * 128
                    op = ps.tile([128, QB], F32, tag="pa",
                                 name=f"op{jb}{st}{nb}", uniquify=True)
                    for kt2 in range(2):
                        nc.tensor.matmul(
                            op[:], packed[:, kt2, s0:s0 + 128],
                            wo_sb[:, kt2, nb * QB:nb * QB + QB],
                            start=(kt2 == 0), stop=(kt2 == 1))
                    ot = sbO.tile([128, QB], F32, tag="ot")
                    nc.vector.tensor_copy(ot[:], op[:])
                    nc.sync.dma_start(
                        out[s0:s0 + 128, nb * QB:nb * QB + QB], ot[:])
                return go

            for st in range(QB // 128):
                for nb in range(2):
                    fl.append(wostep(st, nb))
            return fl

        fillers = deque()

        def drain(n):
            for _ in range(n):
                if not fillers:
                    return
                fillers.popleft()()

        # chunk 0 projections run standalone (pipeline head)
        for f in proj_fillers(0):
            f()

        for ch in range(NCH):
            jb = ch
            q0 = jb * QB
            nkt = 4 * (jb + 1)
            # prefetch x chunks, queue next projections + previous Wo
            if ch + 2 < NCH:
                nc.sync.dma_start(xcs[ch + 2][:],
                                  xTr[:, :, (ch + 2) * QB:(ch + 3) * QB])
            if ch + 1 < NCH:
                fillers.extend(proj_fillers(ch + 1))
            if ch >= 1:
                fillers.extend(wo_fillers(ch - 1))
            steps_left = 2 * nkt
            for pair in range(2):
                cps = [ps.tile([128, QB], F32, tag="cp", name=f"cp{jb}{pair}{hh}")
                       for hh in range(2)]

                def score_step(kt_i):
                    """scores + exp + mask for one k tile (one step ahead of
                    the ctx matmuls so the PE never sits behind ScalarE)."""
                    cc = max(0, (kt_i - 4 * jb) * KT)
                    sp = ps.tile([128, 2, QB], F32, tag="sp", name="sp",
                                 uniquify=True)
                    for hh in range(2):
                        p0 = hh * DH
                        nc.tensor.matmul(
                            sp[:, hh, cc:],
                            kT[p0:p0 + DH, kt_i * KT:kt_i * KT + KT],
                            qT2[p0:p0 + DH, pair, q0 + cc:q0 + QB],
                            start=True, stop=True)
                    pt = sbP.tile([128, 2, QB], BF, tag="pt", name="pt",
                                  uniquify=True)
                    nc.scalar.activation(pt[:, :, cc:], sp[:, :, cc:], EXP,
                                         scale=0.125)
                    if kt_i >= 4 * jb:
                        nc.gpsimd.tensor_tensor(
                            pt[:, :, cc:cc + KT], pt[:, :, cc:cc + KT],
                            mask_sb[:], MULT)
                    return pt

                pts = {0: score_step(0)}
                for kt_i in range(nkt):
                    if kt_i + 1 < nkt:
                        pts[kt_i + 1] = score_step(kt_i + 1)
                    cc = max(0, (kt_i - 4 * jb) * KT)
                    pt = pts.pop(kt_i)
                    for hh in range(2):
                        nc.tensor.matmul(
                            cps[hh][:, cc:], vt[kt_i][:], pt[:, hh, cc:],
                            start=(kt_i == 0), stop=(kt_i == nkt - 1))
                    # keep the PE busy while ScalarE runs exp
                    drain(-(-len(fillers) // steps_left))
                    steps_left -= 1
                # normalize: 1/denom (rows 64:128) * ctx (rows 0:64)
                for hh in range(2):
                    h = 2 * pair + hh
                    dns = sbN.tile([DH, QB], F32, tag="dns")
                    nc.vector.tensor_copy(dns[:], cps[hh][DH:128, :])
                    rb = sbN.tile([DH, QB], F32, tag="rb")
                    nc.vector.reciprocal_approx_fast(rb[:], dns[:])
                    p0 = (h % 2) * DH
                    nc.vector.tensor_tensor(
                        packed[p0:p0 + DH, h // 2, q0:q0 + QB],
                        cps[hh][0:DH, :], rb[:], MULT)
            drain(len(fillers))
        for f in wo_fillers(NCH - 1):
            f()
    nc.compile()
    return nc


def _rope_tables():
    inv = 1.0 / (10000.0 ** (np.arange(0, DH, 2, dtype=np.float32) / DH))
    t = np.arange(S, dtype=np.float32)
    freqs = np.outer(t, inv).astype(np.float32)          # [S, 32]
    emb = np.concatenate([freqs, freqs], axis=1)         # [S, 64]
    cosT = np.cos(emb).T.astype(np.float32)              # [64, S]
    sinT = np.sin(emb).T.astype(np.float32)
    return (np.concatenate([cosT, cosT], 0).copy(),
            np.concatenate([sinT, sinT], 0).copy())


def _mask2():
    r = np.arange(128)[:, None]
    c = np.arange(128)[None, :]
    m = (r <= c).astype(np.float32)
    return np.broadcast_to(m[:, None, :], (128, 2, 128)).astype(BF_NP).copy()


_NC_CACHE = {}


def _get_nc():
    if "nc" not in _NC_CACHE:
        _NC_CACHE["nc"] = build_bass()
    return _NC_CACHE["nc"]


def run(x, Wq, Wk, Wv, Wo, trace=False):
    nc = _get_nc()
    cos2, sin2 = _rope_tables()
    ident = np.eye(128, dtype=np.float32).astype(BF_NP)
    mask = _mask2()
    in_maps = []
    for c in range(NCHIPS):
        b, g = c // KV, c % KV
        in_maps.append({
            "xT": np.ascontiguousarray(x[b].T).astype(BF_NP),
            "wq": np.ascontiguousarray(Wq[:, g * DG:(g + 1) * DG]).astype(BF_NP),
            "wk": np.ascontiguousarray(Wk[:, g * DH:(g + 1) * DH]).astype(BF_NP),
            "wv": np.ascontiguousarray(Wv[:, g * DH:(g + 1) * DH]).astype(BF_NP),
            "wo": np.ascontiguousarray(Wo[g * DG:(g + 1) * DG, :]).astype(BF_NP),
            "cos2": cos2, "sin2": sin2, "ident": ident, "mask2": mask,
        })
    res = bass_utils.run_bass_kernel_spmd(
        nc, in_maps, core_ids=list(range(NCHIPS)), trace=trace)
    outs = [np.asarray(r["out"], dtype=np.float32) for r in res.results]
    full = np.zeros((B, S, DM), dtype=np.float32)
    for c in range(NCHIPS):
        full[c // KV] += outs[c]
    return full, res


def kernel(x, Wq, Wk, Wv, Wo):
    full, _ = run(np.asarray(x, dtype=np.float32), np.asarray(Wq),
                  np.asarray(Wk), np.asarray(Wv), np.asarray(Wo))
    return full
